# revision 20
# baseline (speedup 1.0000x reference)
"""Distributed Bass kernel for tied-row MSA attention on 8 TRN2 NeuronCores.

Sharding: batch (the 64 MSA rows) split 8 ways -> 8 rows/core; weights and
attn_bias replicated. The tie_dim mean over q becomes: local row-sum of x,
AllReduce across cores, then (sum/64) @ Wq * dh^-0.5 computed redundantly.

Per-core dataflow (transposed-activation pipeline, bf16 matmuls, f32 softmax):
  x_nat [tok,D] --PE transpose--> xT [D,tok] (bf16)
  rowsumT = sum_r xT (f32) --AllReduce--> q_tiedT = Wq^T @ rowsumT * scale
  kT = Wk^T @ xT (transposed, (h d) on partitions, bf16)
  gT = sigmoid(Wg^T @ xT + bg) (transposed, f32)
  v_nat = xT^T @ Wv (natural, bf16)
  per (r,h): dotsT[j,i] = biasT (identity-matmul PSUM init) + kT_h^T q_tiedT_h
             (dotsT+30)*pair_mask then exp(.-30) on ACT -> attn_exp (bf16)
             bankV[32h'..] = v^T @ attn_exp ; bankS = ones^T @ attn_exp
  out_gatedT = (bankV * gT) / bankS  (bf16), final = out_gatedT^T @ Wo + bo.
"""

import numpy as np

HEADS = 8
DH = 32
B = 64
N = 256
D = 256
INNER = 256
NCORES = 8
RLOC = B // NCORES          # 8 MSA rows per core
T = RLOC * N                # 2048 tokens per core
SCALE_F = 1.0 / (B * (DH ** 0.5))  # tie-mean (1/64) * dh^-0.5, folded into q
MASK_NEG = 30.0             # pre-softmax mask offset

_CACHE = {}


def _build():
    import concourse.bass as bass
    import concourse.mybir as mybir
    import concourse.tile as tile
    from concourse import bacc
    from concourse.masks import make_identity
    from contextlib import ExitStack

    f32 = mybir.dt.float32
    bf16 = mybir.dt.bfloat16
    u8 = mybir.dt.uint8
    AF = mybir.ActivationFunctionType
    ALU = mybir.AluOpType

    nc = bacc.Bacc("TRN2", target_bir_lowering=False, debug=False,
                   num_devices=NCORES)

    x_e = nc.dram_tensor("x", [RLOC, N, D], f32, kind="ExternalInput")
    mask_e = nc.dram_tensor("mask", [RLOC, N], u8, kind="ExternalInput")
    bias_e = nc.dram_tensor("attn_bias", [HEADS, N, N], f32, kind="ExternalInput")
    wq_e = nc.dram_tensor("Wq", [D, INNER], f32, kind="ExternalInput")
    wkv_e = nc.dram_tensor("Wkv", [D, 2 * INNER], f32, kind="ExternalInput")
    wg_e = nc.dram_tensor("Wg", [D, INNER], f32, kind="ExternalInput")
    bg_e = nc.dram_tensor("bg", [INNER], f32, kind="ExternalInput")
    wo_e = nc.dram_tensor("Wo", [INNER, D], f32, kind="ExternalInput")
    bo_e = nc.dram_tensor("bo", [D], f32, kind="ExternalInput")
    out_e = nc.dram_tensor("out", [RLOC, N, D], f32, kind="ExternalOutput")

    with tile.TileContext(nc) as tc, ExitStack() as ctx:
        const = ctx.enter_context(tc.tile_pool(name="const", bufs=1))
        big = ctx.enter_context(tc.tile_pool(name="big", bufs=1))
        work = ctx.enter_context(tc.tile_pool(name="work", bufs=3))
        aexp_pool = ctx.enter_context(tc.tile_pool(name="aexp", bufs=4))
        og_pool = ctx.enter_context(tc.tile_pool(name="og", bufs=4))
        ps_sp = ctx.enter_context(tc.tile_pool(name="ps_sp", bufs=2, space="PSUM"))
        ps_dots = ctx.enter_context(tc.tile_pool(name="ps_dots", bufs=2, space="PSUM"))
        ps_av = ctx.enter_context(tc.tile_pool(name="ps_av", bufs=1, space="PSUM"))
        dram = ctx.enter_context(tc.tile_pool(name="dram", bufs=1, space="DRAM"))

        # ---- constants / weights ----
        ident = const.tile([128, 128], f32)
        make_identity(nc, ident)
        ident_b = const.tile([128, 128], bf16)
        nc.vector.tensor_copy(ident_b, ident)
        ones32 = const.tile([128, 32], bf16)
        nc.vector.memset(ones32, 1.0)
        negm = const.tile([128, 1], f32)
        nc.vector.memset(negm, -MASK_NEG)

        # ---- x load + transpose ----
        x_nat = big.tile([128, 16, D], f32)
        x_flat = x_e.ap().rearrange("r n d -> (r n) d").rearrange("(t p) d -> p t d", p=128)
        for t in range(16):
            nc.sync.dma_start(out=x_nat[:, t, :], in_=x_flat[:, t, :])
        xT = big.tile([128, 2, T], bf16)
        for t in range(16):
            for c in range(2):
                tp = ps_sp.tile([128, 512], f32, tag="sp", name="tp")
                nc.tensor.transpose(tp[:, 0:128], x_nat[:, t, 128 * c:128 * (c + 1)], ident)
                nc.any.tensor_copy(xT[:, c, 128 * t:128 * (t + 1)], tp[:, 0:128])

        # ---- local row-sum of x (transposed, f32 accumulate) ----
        rs = big.tile([128, 2, N], f32)
        for c in range(2):
            s01 = work.tile([128, N], f32, tag="rsw")
            s23 = work.tile([128, N], f32, tag="rsw")
            s45 = work.tile([128, N], f32, tag="rsw")
            s67 = work.tile([128, N], f32, tag="rsw")
            xc = xT[:, c, :]
            nc.vector.tensor_add(s01, xc[:, 0 * N:1 * N], xc[:, 1 * N:2 * N])
            nc.vector.tensor_add(s23, xc[:, 2 * N:3 * N], xc[:, 3 * N:4 * N])
            nc.vector.tensor_add(s45, xc[:, 4 * N:5 * N], xc[:, 5 * N:6 * N])
            nc.vector.tensor_add(s67, xc[:, 6 * N:7 * N], xc[:, 7 * N:8 * N])
            nc.vector.tensor_add(s01, s01, s23)
            nc.vector.tensor_add(s45, s45, s67)
            nc.vector.tensor_add(rs[:, c, :], s01, s45)

        # ---- AllReduce of rowsum ----
        ar_in = dram.tile([2, 128, N], f32)
        ar_out = dram.tile([2, 128, N], f32)
        for c in range(2):
            nc.gpsimd.dma_start(out=ar_in[c], in_=rs[:, c, :])
        nc.gpsimd.collective_compute(
            "AllReduce",
            mybir.AluOpType.add,
            replica_groups=[list(range(NCORES))],
            ins=[ar_in.opt()],
            outs=[ar_out.opt()],
        )
        rsg = big.tile([128, 2, N], f32)
        for c in range(2):
            nc.gpsimd.dma_start(out=rsg[:, c, :], in_=ar_out[c])
        rsg_b = big.tile([128, 2, N], bf16)
        nc.vector.tensor_copy(rsg_b, rsg)

        wq_f = const.tile([128, 2, INNER], f32)
        wk_f = const.tile([128, 2, INNER], f32)
        wv_f = const.tile([128, 2, INNER], f32)
        wg_f = const.tile([128, 2, INNER], f32)
        wo_f = const.tile([128, 2, D], f32)
        for c in range(2):
            nc.sync.dma_start(out=wq_f[:, c, :], in_=wq_e[128 * c:128 * (c + 1), :])
            nc.sync.dma_start(out=wk_f[:, c, :], in_=wkv_e[128 * c:128 * (c + 1), 0:INNER])
            nc.sync.dma_start(out=wv_f[:, c, :], in_=wkv_e[128 * c:128 * (c + 1), INNER:2 * INNER])
            nc.sync.dma_start(out=wg_f[:, c, :], in_=wg_e[128 * c:128 * (c + 1), :])
            nc.sync.dma_start(out=wo_f[:, c, :], in_=wo_e[128 * c:128 * (c + 1), :])
        wq_sb = const.tile([128, 2, INNER], bf16)
        wk_sb = const.tile([128, 2, INNER], bf16)
        wv_sb = const.tile([128, 2, INNER], bf16)
        wg_sb = const.tile([128, 2, INNER], bf16)
        wo_sb = const.tile([128, 2, D], bf16)
        nc.vector.tensor_copy(wq_sb, wq_f)
        nc.vector.tensor_copy(wk_sb, wk_f)
        nc.vector.tensor_copy(wv_sb, wv_f)
        nc.vector.tensor_copy(wg_sb, wg_f)
        nc.vector.tensor_copy(wo_sb, wo_f)
        bg_sb = const.tile([128, 2], f32)
        nc.sync.dma_start(out=bg_sb, in_=bg_e.ap().rearrange("(c p) -> p c", p=128))
        bo_bc = const.tile([128, D], f32)
        nc.sync.dma_start(out=bo_bc, in_=bo_e.ap()[None, :].broadcast_to([128, D]))

        # mask as bf16 on partition 0: [1, (r n)]
        mask_u8 = const.tile([1, T], u8)
        nc.sync.dma_start(out=mask_u8, in_=mask_e.ap().rearrange("r n -> (r n)")[None, :])
        maskf = const.tile([1, T], bf16)
        nc.vector.tensor_copy(maskf, mask_u8)

        # ---- projections (overlap with the collective) ----
        kT = big.tile([128, 2, T], bf16)
        gT = big.tile([128, 2, T], f32)
        for mc in range(2):
            for t4 in range(4):
                kp = ps_sp.tile([128, 512], f32, tag="sp", name="kp")
                for kc in range(2):
                    nc.tensor.matmul(
                        kp, wk_sb[:, kc, 128 * mc:128 * (mc + 1)],
                        xT[:, kc, 512 * t4:512 * (t4 + 1)],
                        start=(kc == 0), stop=(kc == 1))
                nc.any.tensor_copy(kT[:, mc, 512 * t4:512 * (t4 + 1)], kp)
            for t4 in range(4):
                gp = ps_sp.tile([128, 512], f32, tag="sp", name="gp")
                for kc in range(2):
                    nc.tensor.matmul(
                        gp, wg_sb[:, kc, 128 * mc:128 * (mc + 1)],
                        xT[:, kc, 512 * t4:512 * (t4 + 1)],
                        start=(kc == 0), stop=(kc == 1))
                nc.scalar.activation(gT[:, mc, 512 * t4:512 * (t4 + 1)], gp,
                                     AF.Sigmoid, bias=bg_sb[:, mc:mc + 1], scale=1.0)
        v_nat = big.tile([128, 16, INNER], bf16)
        for t in range(16):
            vp = ps_sp.tile([128, 512], f32, tag="sp", name="vp")
            for kc in range(2):
                nc.tensor.matmul(
                    vp[:, 0:256], xT[:, kc, 128 * t:128 * (t + 1)],
                    wv_sb[:, kc, :],
                    start=(kc == 0), stop=(kc == 1))
            nc.any.tensor_copy(v_nat[:, t, :], vp[:, 0:256])

        # ---- attn_bias transposed: biasT [j, (jc h i)] (bf16) ----
        biasT = big.tile([128, 2, HEADS, N], bf16)
        for h in range(HEADS):
            bn = work.tile([128, 2, N], f32, tag="bnat")
            nc.sync.dma_start(
                out=bn, in_=bias_e[h].rearrange("(ic p) j -> p ic j", p=128))
            for ic in range(2):
                for jc in range(2):
                    tp2 = ps_sp.tile([128, 512], f32, tag="sp", name="tp2")
                    nc.tensor.transpose(tp2[:, 0:128], bn[:, ic, 128 * jc:128 * (jc + 1)], ident)
                    nc.any.tensor_copy(biasT[:, jc, h, 128 * ic:128 * (ic + 1)], tp2[:, 0:128])

        # ---- pairwise mask outer products pmT[j, (r jc) i] (f32) ----
        pmT = big.tile([128, 16, N], f32)
        for r in range(RLOC):
            for jc in range(2):
                mp = ps_sp.tile([128, 512], f32, tag="sp", name="mp")
                nc.tensor.matmul(
                    mp[:, 0:N],
                    maskf[0:1, r * N + 128 * jc: r * N + 128 * (jc + 1)],
                    maskf[0:1, r * N:(r + 1) * N],
                    start=True, stop=True)
                nc.any.tensor_copy(pmT[:, 2 * r + jc, :], mp[:, 0:N])

        # ---- q_tiedT from the all-reduced rowsum (bf16) ----
        q_tT = big.tile([128, 2, N], bf16)
        for mc in range(2):
            qp = ps_sp.tile([128, 512], f32, tag="sp", name="qp")
            for kc in range(2):
                nc.tensor.matmul(
                    qp[:, 0:N], wq_sb[:, kc, 128 * mc:128 * (mc + 1)],
                    rsg_b[:, kc, :],
                    start=(kc == 0), stop=(kc == 1))
            nc.vector.tensor_scalar_mul(q_tT[:, mc, :], qp[:, 0:N], SCALE_F)
        # block-diagonal q for full-K dots matmuls: qbd[hg][half] [128, 512]
        # rows 32*hp hold head (4hg+2*half+hh) q at free hh*N.. ; rest zero
        qbd = big.tile([128, 2, 2, 512], bf16)
        nc.vector.memset(qbd, 0.0)
        for hg in range(2):
            for half in range(2):
                for hh in range(2):
                    hp = 2 * half + hh
                    nc.vector.tensor_copy(
                        qbd[32 * hp:32 * (hp + 1), hg, half, N * hh:N * (hh + 1)],
                        q_tT[32 * hp:32 * (hp + 1), hg, :])

        # ---- attention ----
        for r in range(RLOC):
            aexps = {}
            for jc in range(2):
                for hg in range(2):
                    mega = ps_dots.tile([128, 4 * N], f32, tag="dots", name="mega")
                    for half in range(2):
                        sl = slice(512 * half, 512 * (half + 1))
                        nc.tensor.matmul(
                            mega[:, sl], ident_b,
                            biasT[:, jc, 4 * hg + 2 * half:4 * hg + 2 * half + 2, :],
                            start=True, stop=False)
                        nc.tensor.matmul(
                            mega[:, sl],
                            kT[:, hg, r * N + 128 * jc: r * N + 128 * (jc + 1)],
                            qbd[:, hg, half, :],
                            start=False, stop=True)
                    et = work.tile([128, 4 * N], f32, tag="et")
                    nc.vector.scalar_tensor_tensor(
                        out=et.rearrange("p (s n) -> p s n", s=4),
                        in0=mega.rearrange("p (s n) -> p s n", s=4),
                        scalar=MASK_NEG,
                        in1=pmT[:, 2 * r + jc:2 * r + jc + 1, :].broadcast_to([128, 4, N]),
                        op0=ALU.add, op1=ALU.mult)
                    ae = aexp_pool.tile([128, 4 * N], bf16, tag="ae")
                    nc.scalar.activation(ae, et, AF.Exp, bias=negm, scale=1.0)
                    aexps[(jc, hg)] = ae

            ogs = {}
            for hg in range(2):
                bankV = ps_av.tile([128, N], f32, tag="bv", name="bankV")
                bankS = ps_av.tile([128, N], f32, tag="bs", name="bankS")
                for hp in range(4):
                    h = 4 * hg + hp
                    orow = slice(32 * hp, 32 * (hp + 1))
                    for jc in range(2):
                        rhs = aexps[(jc, hg)][:, N * hp:N * (hp + 1)]
                        nc.tensor.matmul(
                            bankV[orow, :],
                            v_nat[:, 2 * r + jc, 32 * h:32 * (h + 1)],
                            rhs, start=(jc == 0), stop=(jc == 1),
                            tile_position=(0, 32 * hp))
                        nc.tensor.matmul(
                            bankS[orow, :], ones32, rhs,
                            start=(jc == 0), stop=(jc == 1),
                            tile_position=(0, 32 * hp))
                tgv = work.tile([128, N], f32, tag="tgv")
                nc.vector.tensor_mul(tgv, bankV, gT[:, hg, r * N:(r + 1) * N])
                rcs = work.tile([128, N], f32, tag="rcs")
                nc.vector.reciprocal(rcs, bankS)
                og = og_pool.tile([128, N], bf16, tag="og", name="og")
                nc.vector.tensor_mul(og, tgv, rcs)
                ogs[hg] = og

            # final = out_gatedT^T @ Wo  (natural layout out) + bo
            for ic in range(2):
                fp = ps_sp.tile([128, 512], f32, tag="sp", name="fp")
                nc.tensor.matmul(fp[:, 0:D], ogs[0][:, 128 * ic:128 * (ic + 1)],
                                 wo_sb[:, 0, :], start=True, stop=False)
                nc.tensor.matmul(fp[:, 0:D], ogs[1][:, 128 * ic:128 * (ic + 1)],
                                 wo_sb[:, 1, :], start=False, stop=True)
                fo = work.tile([128, D], f32, tag="fo")
                nc.vector.tensor_add(fo, fp[:, 0:D], bo_bc)
                nc.sync.dma_start(out=out_e[r, 128 * ic:128 * (ic + 1), :], in_=fo)

    nc.finalize()
    return nc


def _get_nc():
    if "nc" not in _CACHE:
        _CACHE["nc"] = _build()
    return _CACHE["nc"]


def _in_maps(x, mask, attn_bias, Wq, Wkv, Wg, bg, Wo, bo):
    shared = {
        "attn_bias": np.ascontiguousarray(np.asarray(attn_bias).reshape(HEADS, N, N), dtype=np.float32),
        "Wq": np.ascontiguousarray(Wq, dtype=np.float32),
        "Wkv": np.ascontiguousarray(Wkv, dtype=np.float32),
        "Wg": np.ascontiguousarray(Wg, dtype=np.float32),
        "bg": np.ascontiguousarray(bg, dtype=np.float32),
        "Wo": np.ascontiguousarray(Wo, dtype=np.float32),
        "bo": np.ascontiguousarray(bo, dtype=np.float32),
    }
    maps = []
    for c in range(NCORES):
        sh = slice(c * RLOC, (c + 1) * RLOC)
        m = dict(shared)
        m["x"] = np.ascontiguousarray(np.asarray(x)[sh], dtype=np.float32)
        m["mask"] = np.ascontiguousarray(np.asarray(mask)[sh]).astype(np.uint8)
        maps.append(m)
    return maps


def kernel(x, mask, attn_bias, Wq, Wkv, Wg, bg, Wo, bo, tie_dim=64, **_unused):
    from concourse.bass_utils import run_bass_kernel_spmd

    nc = _get_nc()
    in_maps = _in_maps(x, mask, attn_bias, Wq, Wkv, Wg, bg, Wo, bo)
    res = run_bass_kernel_spmd(nc, in_maps, core_ids=list(range(NCORES)))
    out = np.concatenate([res.results[i]["out"] for i in range(NCORES)], axis=0)
    return out.reshape(B, N, D).astype(np.float32)


# revision 21
# speedup vs baseline: 1.0043x; 1.0043x over previous
"""Distributed Bass kernel for tied-row MSA attention on 8 TRN2 NeuronCores.

Sharding: batch (the 64 MSA rows) split 8 ways -> 8 rows/core; weights and
attn_bias replicated. The tie_dim mean over q becomes: local row-sum of x,
AllReduce across cores, then (sum/64) @ Wq * dh^-0.5 computed redundantly.

Per-core dataflow (transposed-activation pipeline, bf16 matmuls, f32 softmax):
  x_nat [tok,D] --PE transpose--> xT [D,tok] (bf16)
  rowsumT = sum_r xT (f32) --AllReduce--> q_tiedT = Wq^T @ rowsumT * scale
  kT = Wk^T @ xT (transposed, (h d) on partitions, bf16)
  gT = sigmoid(Wg^T @ xT + bg) (transposed, f32)
  v_nat = xT^T @ Wv (natural, bf16)
  per (r,h): dotsT[j,i] = biasT (identity-matmul PSUM init) + kT_h^T q_tiedT_h
             (dotsT+30)*pair_mask then exp(.-30) on ACT -> attn_exp (bf16)
             bankV[32h'..] = v^T @ attn_exp ; bankS = ones^T @ attn_exp
  out_gatedT = (bankV * gT) / bankS  (bf16), final = out_gatedT^T @ Wo + bo.
"""

import numpy as np

HEADS = 8
DH = 32
B = 64
N = 256
D = 256
INNER = 256
NCORES = 8
RLOC = B // NCORES          # 8 MSA rows per core
T = RLOC * N                # 2048 tokens per core
SCALE_F = 1.0 / (B * (DH ** 0.5))  # tie-mean (1/64) * dh^-0.5, folded into q
MASK_NEG = 30.0             # pre-softmax mask offset

_CACHE = {}


def _build():
    import concourse.bass as bass
    import concourse.mybir as mybir
    import concourse.tile as tile
    from concourse import bacc
    from concourse.masks import make_identity
    from contextlib import ExitStack

    f32 = mybir.dt.float32
    bf16 = mybir.dt.bfloat16
    u8 = mybir.dt.uint8
    AF = mybir.ActivationFunctionType
    ALU = mybir.AluOpType

    nc = bacc.Bacc("TRN2", target_bir_lowering=False, debug=False,
                   num_devices=NCORES)

    x_e = nc.dram_tensor("x", [RLOC, N, D], f32, kind="ExternalInput")
    mask_e = nc.dram_tensor("mask", [RLOC, N], u8, kind="ExternalInput")
    bias_e = nc.dram_tensor("attn_bias", [HEADS, N, N], f32, kind="ExternalInput")
    wq_e = nc.dram_tensor("Wq", [D, INNER], f32, kind="ExternalInput")
    wkv_e = nc.dram_tensor("Wkv", [D, 2 * INNER], f32, kind="ExternalInput")
    wg_e = nc.dram_tensor("Wg", [D, INNER], f32, kind="ExternalInput")
    bg_e = nc.dram_tensor("bg", [INNER], f32, kind="ExternalInput")
    wo_e = nc.dram_tensor("Wo", [INNER, D], f32, kind="ExternalInput")
    bo_e = nc.dram_tensor("bo", [D], f32, kind="ExternalInput")
    out_e = nc.dram_tensor("out", [RLOC, N, D], f32, kind="ExternalOutput")

    with tile.TileContext(nc) as tc, ExitStack() as ctx:
        const = ctx.enter_context(tc.tile_pool(name="const", bufs=1))
        big = ctx.enter_context(tc.tile_pool(name="big", bufs=1))
        work = ctx.enter_context(tc.tile_pool(name="work", bufs=3))
        aexp_pool = ctx.enter_context(tc.tile_pool(name="aexp", bufs=4))
        og_pool = ctx.enter_context(tc.tile_pool(name="og", bufs=4))
        ps_sp = ctx.enter_context(tc.tile_pool(name="ps_sp", bufs=2, space="PSUM"))
        ps_dots = ctx.enter_context(tc.tile_pool(name="ps_dots", bufs=2, space="PSUM"))
        ps_av = ctx.enter_context(tc.tile_pool(name="ps_av", bufs=1, space="PSUM"))
        dram = ctx.enter_context(tc.tile_pool(name="dram", bufs=1, space="DRAM"))

        # ---- constants / weights ----
        ident = const.tile([128, 128], f32)
        make_identity(nc, ident)
        ident_b = const.tile([128, 128], bf16)
        nc.vector.tensor_copy(ident_b, ident)
        ones32 = const.tile([128, 32], bf16)
        nc.vector.memset(ones32, 1.0)
        negm = const.tile([128, 1], f32)
        nc.vector.memset(negm, -MASK_NEG)

        # ---- x load + transpose ----
        x_nat = big.tile([128, 16, D], f32)
        x_flat = x_e.ap().rearrange("r n d -> (r n) d").rearrange("(t p) d -> p t d", p=128)
        for t in range(16):
            nc.sync.dma_start(out=x_nat[:, t, :], in_=x_flat[:, t, :])
        # ---- local row-sum of x via accumulating PE transposes ----
        # rs[dchunk, nchunk] = sum_r transpose(x_nat[:, t(r,nchunk), dchunk])
        # (depends only on x_nat tiles -> the AllReduce fires early)
        rs = big.tile([128, 2, N], f32)
        for c in range(2):
            for nc2 in range(2):
                rp = ps_sp.tile([128, 512], f32, tag="sp", name="rp")
                for r8 in range(8):
                    t = 2 * r8 + nc2
                    nc.tensor.matmul(rp[:, 0:128], x_nat[:, t, 128 * c:128 * (c + 1)],
                                     ident, is_transpose=True,
                                     start=(r8 == 0), stop=(r8 == 7))
                nc.any.tensor_copy(rs[:, c, 128 * nc2:128 * (nc2 + 1)], rp[:, 0:128])

        # ---- AllReduce of rowsum ----
        ar_in = dram.tile([2, 128, N], f32)
        ar_out = dram.tile([2, 128, N], f32)
        for c in range(2):
            nc.gpsimd.dma_start(out=ar_in[c], in_=rs[:, c, :])
        nc.gpsimd.collective_compute(
            "AllReduce",
            mybir.AluOpType.add,
            replica_groups=[list(range(NCORES))],
            ins=[ar_in.opt()],
            outs=[ar_out.opt()],
        )
        rsg = big.tile([128, 2, N], f32)
        for c in range(2):
            nc.gpsimd.dma_start(out=rsg[:, c, :], in_=ar_out[c])
        rsg_b = big.tile([128, 2, N], bf16)
        nc.vector.tensor_copy(rsg_b, rsg)

        wq_f = const.tile([128, 2, INNER], f32)
        wk_f = const.tile([128, 2, INNER], f32)
        wv_f = const.tile([128, 2, INNER], f32)
        wg_f = const.tile([128, 2, INNER], f32)
        wo_f = const.tile([128, 2, D], f32)
        for c in range(2):
            nc.sync.dma_start(out=wq_f[:, c, :], in_=wq_e[128 * c:128 * (c + 1), :])
            nc.sync.dma_start(out=wk_f[:, c, :], in_=wkv_e[128 * c:128 * (c + 1), 0:INNER])
            nc.sync.dma_start(out=wv_f[:, c, :], in_=wkv_e[128 * c:128 * (c + 1), INNER:2 * INNER])
            nc.sync.dma_start(out=wg_f[:, c, :], in_=wg_e[128 * c:128 * (c + 1), :])
            nc.sync.dma_start(out=wo_f[:, c, :], in_=wo_e[128 * c:128 * (c + 1), :])
        wq_sb = const.tile([128, 2, INNER], bf16)
        wk_sb = const.tile([128, 2, INNER], bf16)
        wv_sb = const.tile([128, 2, INNER], bf16)
        wg_sb = const.tile([128, 2, INNER], bf16)
        wo_sb = const.tile([128, 2, D], bf16)
        nc.vector.tensor_copy(wq_sb, wq_f)
        nc.vector.tensor_copy(wk_sb, wk_f)
        nc.vector.tensor_copy(wv_sb, wv_f)
        nc.vector.tensor_copy(wg_sb, wg_f)
        nc.vector.tensor_copy(wo_sb, wo_f)
        bg_sb = const.tile([128, 2], f32)
        nc.sync.dma_start(out=bg_sb, in_=bg_e.ap().rearrange("(c p) -> p c", p=128))
        bo_bc = const.tile([128, D], f32)
        nc.sync.dma_start(out=bo_bc, in_=bo_e.ap()[None, :].broadcast_to([128, D]))

        # mask as bf16 on partition 0: [1, (r n)]
        mask_u8 = const.tile([1, T], u8)
        nc.sync.dma_start(out=mask_u8, in_=mask_e.ap().rearrange("r n -> (r n)")[None, :])
        maskf = const.tile([1, T], bf16)
        nc.vector.tensor_copy(maskf, mask_u8)

        # ---- xT transposes (overlap with the collective) ----
        xT = big.tile([128, 2, T], bf16)
        for t in range(16):
            for c in range(2):
                tp = ps_sp.tile([128, 512], f32, tag="sp", name="tp")
                nc.tensor.transpose(tp[:, 0:128], x_nat[:, t, 128 * c:128 * (c + 1)], ident)
                nc.any.tensor_copy(xT[:, c, 128 * t:128 * (t + 1)], tp[:, 0:128])

        # ---- projections (overlap with the collective) ----
        kT = big.tile([128, 2, T], bf16)
        gT = big.tile([128, 2, T], f32)
        for mc in range(2):
            for t4 in range(4):
                kp = ps_sp.tile([128, 512], f32, tag="sp", name="kp")
                for kc in range(2):
                    nc.tensor.matmul(
                        kp, wk_sb[:, kc, 128 * mc:128 * (mc + 1)],
                        xT[:, kc, 512 * t4:512 * (t4 + 1)],
                        start=(kc == 0), stop=(kc == 1))
                nc.any.tensor_copy(kT[:, mc, 512 * t4:512 * (t4 + 1)], kp)
            for t4 in range(4):
                gp = ps_sp.tile([128, 512], f32, tag="sp", name="gp")
                for kc in range(2):
                    nc.tensor.matmul(
                        gp, wg_sb[:, kc, 128 * mc:128 * (mc + 1)],
                        xT[:, kc, 512 * t4:512 * (t4 + 1)],
                        start=(kc == 0), stop=(kc == 1))
                nc.scalar.activation(gT[:, mc, 512 * t4:512 * (t4 + 1)], gp,
                                     AF.Sigmoid, bias=bg_sb[:, mc:mc + 1], scale=1.0)
        v_nat = big.tile([128, 16, INNER], bf16)
        for t in range(16):
            vp = ps_sp.tile([128, 512], f32, tag="sp", name="vp")
            for kc in range(2):
                nc.tensor.matmul(
                    vp[:, 0:256], xT[:, kc, 128 * t:128 * (t + 1)],
                    wv_sb[:, kc, :],
                    start=(kc == 0), stop=(kc == 1))
            nc.any.tensor_copy(v_nat[:, t, :], vp[:, 0:256])

        # ---- attn_bias transposed: biasT [j, (jc h i)] (bf16) ----
        biasT = big.tile([128, 2, HEADS, N], bf16)
        for h in range(HEADS):
            bn = work.tile([128, 2, N], f32, tag="bnat")
            nc.sync.dma_start(
                out=bn, in_=bias_e[h].rearrange("(ic p) j -> p ic j", p=128))
            for ic in range(2):
                for jc in range(2):
                    tp2 = ps_sp.tile([128, 512], f32, tag="sp", name="tp2")
                    nc.tensor.transpose(tp2[:, 0:128], bn[:, ic, 128 * jc:128 * (jc + 1)], ident)
                    nc.any.tensor_copy(biasT[:, jc, h, 128 * ic:128 * (ic + 1)], tp2[:, 0:128])

        # ---- pairwise mask outer products pmT[j, (r jc) i] (f32) ----
        pmT = big.tile([128, 16, N], f32)
        for r in range(RLOC):
            for jc in range(2):
                mp = ps_sp.tile([128, 512], f32, tag="sp", name="mp")
                nc.tensor.matmul(
                    mp[:, 0:N],
                    maskf[0:1, r * N + 128 * jc: r * N + 128 * (jc + 1)],
                    maskf[0:1, r * N:(r + 1) * N],
                    start=True, stop=True)
                nc.any.tensor_copy(pmT[:, 2 * r + jc, :], mp[:, 0:N])

        # ---- q_tiedT from the all-reduced rowsum (bf16) ----
        q_tT = big.tile([128, 2, N], bf16)
        for mc in range(2):
            qp = ps_sp.tile([128, 512], f32, tag="sp", name="qp")
            for kc in range(2):
                nc.tensor.matmul(
                    qp[:, 0:N], wq_sb[:, kc, 128 * mc:128 * (mc + 1)],
                    rsg_b[:, kc, :],
                    start=(kc == 0), stop=(kc == 1))
            nc.vector.tensor_scalar_mul(q_tT[:, mc, :], qp[:, 0:N], SCALE_F)
        # block-diagonal q for full-K dots matmuls: qbd[hg][half] [128, 512]
        # rows 32*hp hold head (4hg+2*half+hh) q at free hh*N.. ; rest zero
        qbd = big.tile([128, 2, 2, 512], bf16)
        nc.vector.memset(qbd, 0.0)
        for hg in range(2):
            for half in range(2):
                for hh in range(2):
                    hp = 2 * half + hh
                    nc.vector.tensor_copy(
                        qbd[32 * hp:32 * (hp + 1), hg, half, N * hh:N * (hh + 1)],
                        q_tT[32 * hp:32 * (hp + 1), hg, :])

        # ---- attention ----
        for r in range(RLOC):
            aexps = {}
            for jc in range(2):
                for hg in range(2):
                    mega = ps_dots.tile([128, 4 * N], f32, tag="dots", name="mega")
                    for half in range(2):
                        sl = slice(512 * half, 512 * (half + 1))
                        nc.tensor.matmul(
                            mega[:, sl], ident_b,
                            biasT[:, jc, 4 * hg + 2 * half:4 * hg + 2 * half + 2, :],
                            start=True, stop=False)
                        nc.tensor.matmul(
                            mega[:, sl],
                            kT[:, hg, r * N + 128 * jc: r * N + 128 * (jc + 1)],
                            qbd[:, hg, half, :],
                            start=False, stop=True)
                    et = work.tile([128, 4 * N], f32, tag="et")
                    nc.vector.scalar_tensor_tensor(
                        out=et.rearrange("p (s n) -> p s n", s=4),
                        in0=mega.rearrange("p (s n) -> p s n", s=4),
                        scalar=MASK_NEG,
                        in1=pmT[:, 2 * r + jc:2 * r + jc + 1, :].broadcast_to([128, 4, N]),
                        op0=ALU.add, op1=ALU.mult)
                    ae = aexp_pool.tile([128, 4 * N], bf16, tag="ae")
                    nc.scalar.activation(ae, et, AF.Exp, bias=negm, scale=1.0)
                    aexps[(jc, hg)] = ae

            ogs = {}
            for hg in range(2):
                bankV = ps_av.tile([128, N], f32, tag="bv", name="bankV")
                bankS = ps_av.tile([128, N], f32, tag="bs", name="bankS")
                for hp in range(4):
                    h = 4 * hg + hp
                    orow = slice(32 * hp, 32 * (hp + 1))
                    for jc in range(2):
                        rhs = aexps[(jc, hg)][:, N * hp:N * (hp + 1)]
                        nc.tensor.matmul(
                            bankV[orow, :],
                            v_nat[:, 2 * r + jc, 32 * h:32 * (h + 1)],
                            rhs, start=(jc == 0), stop=(jc == 1),
                            tile_position=(0, 32 * hp))
                        nc.tensor.matmul(
                            bankS[orow, :], ones32, rhs,
                            start=(jc == 0), stop=(jc == 1),
                            tile_position=(0, 32 * hp))
                tgv = work.tile([128, N], f32, tag="tgv")
                nc.vector.tensor_mul(tgv, bankV, gT[:, hg, r * N:(r + 1) * N])
                rcs = work.tile([128, N], f32, tag="rcs")
                nc.vector.reciprocal(rcs, bankS)
                og = og_pool.tile([128, N], bf16, tag="og", name="og")
                nc.vector.tensor_mul(og, tgv, rcs)
                ogs[hg] = og

            # final = out_gatedT^T @ Wo  (natural layout out) + bo
            for ic in range(2):
                fp = ps_sp.tile([128, 512], f32, tag="sp", name="fp")
                nc.tensor.matmul(fp[:, 0:D], ogs[0][:, 128 * ic:128 * (ic + 1)],
                                 wo_sb[:, 0, :], start=True, stop=False)
                nc.tensor.matmul(fp[:, 0:D], ogs[1][:, 128 * ic:128 * (ic + 1)],
                                 wo_sb[:, 1, :], start=False, stop=True)
                fo = work.tile([128, D], f32, tag="fo")
                nc.vector.tensor_add(fo, fp[:, 0:D], bo_bc)
                nc.sync.dma_start(out=out_e[r, 128 * ic:128 * (ic + 1), :], in_=fo)

    nc.finalize()
    return nc


def _get_nc():
    if "nc" not in _CACHE:
        _CACHE["nc"] = _build()
    return _CACHE["nc"]


def _in_maps(x, mask, attn_bias, Wq, Wkv, Wg, bg, Wo, bo):
    shared = {
        "attn_bias": np.ascontiguousarray(np.asarray(attn_bias).reshape(HEADS, N, N), dtype=np.float32),
        "Wq": np.ascontiguousarray(Wq, dtype=np.float32),
        "Wkv": np.ascontiguousarray(Wkv, dtype=np.float32),
        "Wg": np.ascontiguousarray(Wg, dtype=np.float32),
        "bg": np.ascontiguousarray(bg, dtype=np.float32),
        "Wo": np.ascontiguousarray(Wo, dtype=np.float32),
        "bo": np.ascontiguousarray(bo, dtype=np.float32),
    }
    maps = []
    for c in range(NCORES):
        sh = slice(c * RLOC, (c + 1) * RLOC)
        m = dict(shared)
        m["x"] = np.ascontiguousarray(np.asarray(x)[sh], dtype=np.float32)
        m["mask"] = np.ascontiguousarray(np.asarray(mask)[sh]).astype(np.uint8)
        maps.append(m)
    return maps


def kernel(x, mask, attn_bias, Wq, Wkv, Wg, bg, Wo, bo, tie_dim=64, **_unused):
    from concourse.bass_utils import run_bass_kernel_spmd

    nc = _get_nc()
    in_maps = _in_maps(x, mask, attn_bias, Wq, Wkv, Wg, bg, Wo, bo)
    res = run_bass_kernel_spmd(nc, in_maps, core_ids=list(range(NCORES)))
    out = np.concatenate([res.results[i]["out"] for i in range(NCORES)], axis=0)
    return out.reshape(B, N, D).astype(np.float32)
